# revision 1
# baseline (speedup 1.0000x reference)
"""Distributed multi-head attention kernel for Trainium2 (8 NeuronCores).

Reference computation (EMBED=1024, HEADS=16, b=2, n=2048):
    qkv = x @ w_qkv.T                       -> [b, n, h, d, 3] (qkv innermost)
    q, k, v per head; energy = q @ k^T
    att = softmax(energy, -1) / sqrt(1024)
    out = att @ v -> [b, n, 1024]
    relu(out @ w_proj.T + b_proj)

Sharding: 2-way data parallel over batch x 4-way tensor parallel over heads.
Core c handles batch c//4, heads [4*(c%4) .. 4*(c%4)+3].  After attention,
each 4-core batch group AllGathers the per-core attention output features
and every core computes a 256-feature slice of the output projection.

Final design, 292us HW exec (vs the fp32r staged-phase v1 at 462us):
  * fp16 for the q/k path (x, w_qkv, q, k): 1 cycle/col on the PE vs the
    ~1.5-3x passes fp32/fp32r matmuls cost; rel err ~3e-3 (validated on host).
  * bf16 for everything softmax-onward (exp, v, att out, w_proj, AllGather,
    final output): exp needs bf16 range (values up to e^+45).
  * Single fused pipeline: K/V production interleaves with qt=0's attention
    (kt follows nt availability), Q[qt] and proj[qt-1] are emitted inside
    later qt's kt loop as PE gap fillers.  The ACT engine (exp, ~143us of
    [128,1024] activations) is the target critical path; the PE stream is
    kept dense so HAM stays warm.
  * PSUM budget (8 banks): energy pool [128,1024]f32 x2 (4 banks, shared
    with Q/K/V/proj chunk borrows), PV accumulators [65,512]f32 x4 (4 banks,
    live across each qt's kt loop; row 64 = ones-column softmax denominator).
  * Softmax normalization: denominators scatter-DMA'd to [128,8] so the DVE
    reciprocal runs 128-wide (v1 ran it 64x redundant on a broadcast tile:
    53us of DVE), then broadcast back over 64 partitions for one multiply.
  * AllGather in bf16 (halves wire bytes), one per qt (4 total).
"""

import os
import sys
import types

sys.path.insert(0, "/opt/trn_rl_repo")

import numpy as np
import ml_dtypes


def _install_ntff_shim():
    """The agent image's antenv lacks axon_hooks; recreate it so
    run_bass_kernel_spmd(trace=True) can capture NTFF profiles."""
    try:
        import antenv.axon_hooks  # noqa: F401
        return
    except ImportError:
        pass
    try:
        import antenv
        from trn_agent_boot.trn_boot import _ntff_profile_via_ctypes
    except ImportError:
        return
    mod = types.ModuleType("antenv.axon_hooks")
    _hook = [None]
    mod.set_axon_ntff_profile_hook = lambda h: _hook.__setitem__(0, h)
    mod.get_axon_ntff_profile_hook = lambda: _hook[0]
    sys.modules["antenv.axon_hooks"] = mod
    antenv.axon_hooks = mod
    mod.set_axon_ntff_profile_hook(
        _ntff_profile_via_ctypes("/opt/axon/libaxon_pjrt.so")
    )


_install_ntff_shim()

import concourse.bacc as bacc
import concourse.bass as bass
import concourse.tile as tile
from concourse import mybir
from concourse.bass_utils import run_bass_kernel_spmd

B, N, E, H, D = 2, 2048, 1024, 16, 64
NCORES = 8
GROUPS = [[0, 1, 2, 3], [4, 5, 6, 7]]
HPC = H // 4            # heads per core = 4
FC = HPC * D            # attention-output features per core = 256
QKV_F = 3 * FC          # qkv features per core = 768
ET = E // 128           # 8 k-tiles over the embed dim
NT = N // 512           # 4 n-tiles of 512
KT = N // 128           # 16 k-tiles of 128 over sequence
F32 = mybir.dt.float32
F16 = mybir.dt.float16
BF16 = mybir.dt.bfloat16

LAST_EXEC_NS = None
LAST_RESULTS = None

_CACHED_NC = None


def _build():
    nc = bacc.Bacc("TRN2", target_bir_lowering=False, num_devices=NCORES)

    # host-side layouts are pre-transposed so every input load is one fully
    # contiguous DMA burst
    xt_d = nc.dram_tensor("xt", [NT, 128, ET, 512], F16, kind="ExternalInput")
    wqkv_d = nc.dram_tensor("wqkvt", [128, ET, QKV_F], F16, kind="ExternalInput")
    wproj_d = nc.dram_tensor("wprojt", [128, ET, FC], BF16, kind="ExternalInput")
    bias_d = nc.dram_tensor("bias", [FC], F32, kind="ExternalInput")
    out_d = nc.dram_tensor("out", [FC, N], BF16, kind="ExternalOutput")

    with tile.TileContext(nc) as tc:
        with (
            tc.tile_pool(name="persist", bufs=1) as persist,
            tc.tile_pool(name="dram", bufs=1, space="DRAM") as dram,
            tc.tile_pool(name="xtp", bufs=NT) as xtp,
            tc.tile_pool(name="eps", bufs=2, space="PSUM") as eps_pool,
            tc.tile_pool(name="pvps", bufs=4, space="PSUM") as pvps_pool,
            tc.tile_pool(name="expp", bufs=4) as expp,
            tc.tile_pool(name="normp", bufs=2) as normp,
            tc.tile_pool(name="prhs", bufs=2) as prhs_pool,
            tc.tile_pool(name="outp", bufs=2) as outp,
        ):
            # ---- persistent SBUF tensors -------------------------------
            wqkv_sb = persist.tile([128, ET, QKV_F], F16)
            wproj_sb = persist.tile([128, ET, FC], BF16)
            bias_sb = persist.tile([128, 2], F32)

            # tiny warm-up AllGather: absorbs the first-collective rendezvous
            # / ncfw cold cost while the lead-in computes.
            warm_in = dram.tile([1, 64], BF16, name="warm_in")
            warm_out = dram.tile([4, 64], BF16, name="warm_out")
            nc.gpsimd.collective_compute(
                "AllGather",
                mybir.AluOpType.bypass,
                replica_groups=GROUPS,
                ins=[warm_in.opt()],
                outs=[warm_out.opt()],
            )

            # q/k features of head pair p (2 heads x 64d) on partitions
            qt_sb = persist.tile([128, 2, N], F16)
            kt_sb = persist.tile([128, 2, N], F16)
            # v in [n, d] layout + a ones column per head: slot = [64 v | 1]
            v_sb = persist.tile([128, KT, HPC, 65], BF16)
            ones_col = nc.const_aps.tensor(1.0, [128, KT, HPC, 1], F32)
            nc.vector.tensor_copy(v_sb[:, :, :, 64:65], ones_col)

            # DRAM bounce buffers
            ot_ch = [dram.tile([FC, 512], BF16, name=f"ot{i}") for i in range(NT)]
            og_ch = [
                dram.tile([4 * FC, 512], BF16, name=f"og{i}") for i in range(NT)
            ]

            xts = []

            # ---- emitters ---------------------------------------------
            def emit_x_load(nt):
                xt_t = xtp.tile([128, ET, 512], F16, tag="xt")
                xts.append(xt_t)
                nc.sync.dma_start(out=xt_t[:], in_=xt_d[nt])

            def emit_qk(nt, pair, which):
                # which: 0 -> q, 1 -> k
                ps = eps_pool.tile([128, 512], F32, tag="e")
                off = which * 256 + pair * 128
                for kt in range(ET):
                    nc.tensor.matmul(
                        ps[:],
                        lhsT=wqkv_sb[:, kt, off : off + 128],
                        rhs=xts[nt][:, kt, :],
                        start=(kt == 0),
                        stop=(kt == ET - 1),
                    )
                dst = qt_sb if which == 0 else kt_sb
                nc.vector.tensor_copy(
                    dst[:, pair, nt * 512 : (nt + 1) * 512], ps[:]
                )

            def emit_v(nt, m):
                ps = eps_pool.tile([128, FC], F32, tag="e")
                for kt in range(ET):
                    nc.tensor.matmul(
                        ps[:],
                        lhsT=xts[nt][:, kt, m * 128 : (m + 1) * 128],
                        rhs=wqkv_sb[:, kt, 512:768],
                        start=(kt == 0),
                        stop=(kt == ET - 1),
                    )
                nc.vector.tensor_copy(
                    v_sb[:, nt * 4 + m, :, 0:64],
                    ps[:].rearrange("p (h d) -> p h d", h=HPC),
                )

            def attn_slot(qt, kt, pvt):
                # pvt: list of 4 per-head PV psum accumulators [65, 512]
                q_sl = slice(qt * 512, (qt + 1) * 512)
                exps = []
                for pair in range(2):
                    ep = eps_pool.tile([128, 1024], F32, tag="e")
                    for s in range(2):
                        d_sl = slice(s * 64, (s + 1) * 64)
                        nc.tensor.matmul(
                            ep[:, s * 512 : (s + 1) * 512],
                            lhsT=kt_sb[d_sl, pair, kt * 128 : (kt + 1) * 128],
                            rhs=qt_sb[d_sl, pair, q_sl],
                            start=True,
                            stop=True,
                        )
                    ex = expp.tile([128, 1024], BF16, tag="exp")
                    nc.scalar.activation(
                        ex[:], ep[:], mybir.ActivationFunctionType.Exp
                    )
                    exps.append(ex)
                for pair in range(2):
                    for s in range(2):
                        nc.tensor.matmul(
                            pvt[2 * pair + s][0:65, :],
                            lhsT=v_sb[:, kt, 2 * pair + s, :],
                            rhs=exps[pair][:, s * 512 : (s + 1) * 512],
                            start=(kt == 0),
                            stop=(kt == KT - 1),
                        )

            def emit_norm(qt, pvt):
                # normalize out^T[d, q] by 1/denominator[q] and store to the
                # AllGather input chunk.
                # 1) evacuate PV psum to SBUF immediately so the psum banks
                #    free up for the next qt's accumulators.  denominators go
                #    through the DVE (heads the DMA chain ASAP); the big pv
                #    copies run on the otherwise-idle ACT engine in parallel.
                den_sb = normp.tile([1, 2048], F32, tag="den_sb")
                pv_sb = normp.tile([64, 2048], F32, tag="pv_sb")
                for i in range(4):
                    nc.vector.tensor_copy(
                        den_sb[:, i * 512 : (i + 1) * 512], pvt[i][64:65, :]
                    )
                for i in range(4):
                    # after the den rows so the reciprocal DMA chain heads out
                    # first; on DVE so the ACT engine stays free for the next
                    # q-tile's exp
                    nc.vector.tensor_copy(
                        pv_sb[:, i * 512 : (i + 1) * 512], pvt[i][0:64, :]
                    )
                # 2) reciprocal on a [128, 16] scatter so the DVE is
                #    full-width (one DRAM bounce round-trip for all 4 heads)
                den_d = dram.tile([1, 2048], F32, tag="den", bufs=2)
                nc.sync.dma_start(out=den_d[:], in_=den_sb[:])
                den_sc = normp.tile([128, 16], F32, tag="den_sc")
                nc.sync.dma_start(
                    out=den_sc[:],
                    in_=bass.AP(
                        tensor=den_d.tensor,
                        offset=den_d.offset,
                        ap=[[16, 128], [1, 16]],
                    ),
                )
                rec_sc = normp.tile([128, 16], F32, tag="rec_sc")
                nc.vector.reciprocal(rec_sc[:], den_sc[:])
                rec_d = dram.tile([1, 2048], F32, tag="rec", bufs=2)
                nc.sync.dma_start(
                    out=bass.AP(
                        tensor=rec_d.tensor,
                        offset=rec_d.offset,
                        ap=[[16, 128], [1, 16]],
                    ),
                    in_=rec_sc[:],
                )
                rep = normp.tile([64, 2048], F32, tag="rep")
                nc.sync.dma_start(
                    out=rep[:],
                    in_=bass.AP(
                        tensor=rec_d.tensor,
                        offset=rec_d.offset,
                        ap=[[0, 64], [1, 2048]],
                    ),
                )
                # 3) normalize + store: mul and DMA pipelined per pair
                o_sb = normp.tile([64, 2048], BF16, tag="o")
                for pair in range(2):
                    p_sl = slice(pair * 1024, (pair + 1) * 1024)
                    nc.vector.tensor_mul(
                        o_sb[:, p_sl], pv_sb[:, p_sl], rep[:, p_sl]
                    )
                    nc.sync.dma_start(
                        out=ot_ch[qt][
                            2 * pair * 64 : 2 * (pair + 1) * 64, :
                        ].rearrange("(s dd) q -> dd s q", s=2),
                        in_=o_sb[:, p_sl].rearrange("dd (s q) -> dd s q", s=2),
                    )

            def emit_ag(qt):
                nc.gpsimd.collective_compute(
                    "AllGather",
                    mybir.AluOpType.bypass,
                    replica_groups=GROUPS,
                    ins=[ot_ch[qt].opt()],
                    outs=[og_ch[qt].opt()],
                )

            proj_rhs = {}

            def emit_proj_rhs(ch):
                rhs_t = prhs_pool.tile([128, ET, 512], BF16, tag="prhs")
                proj_rhs[ch] = rhs_t
                # gpsimd queue: this DMA waits on the AllGather; keep that
                # wait off the sync queue so norm-chain DMAs never stall
                nc.gpsimd.dma_start(
                    out=rhs_t[:],
                    in_=og_ch[ch][:].rearrange("(k p) n -> p k n", p=128),
                )

            def emit_proj_mg(ch, mg):
                pps = eps_pool.tile([128, 512], F32, tag="e")
                for kt in range(ET):
                    nc.tensor.matmul(
                        pps[:],
                        lhsT=wproj_sb[:, kt, mg * 128 : (mg + 1) * 128],
                        rhs=proj_rhs[ch][:, kt, :],
                        start=(kt == 0),
                        stop=(kt == ET - 1),
                    )
                ob = outp.tile([128, 512], BF16, tag="ob")
                nc.vector.tensor_scalar(
                    ob[:],
                    pps[:],
                    bias_sb[:, mg : mg + 1],
                    0.0,
                    mybir.AluOpType.add,
                    mybir.AluOpType.max,
                )
                nc.sync.dma_start(
                    out=out_d[mg * 128 : (mg + 1) * 128, ch * 512 : (ch + 1) * 512],
                    in_=ob[:],
                )

            # ---- fused schedule ---------------------------------------
            # lead-in: per n-tile produce K,V (and Q for qt0), and run qt0's
            # attention k-tiles as soon as their K/V exist.
            pv_tiles = {}

            def alloc_pv(qt):
                pv_tiles[qt] = [
                    pvps_pool.tile([65, 512], F32, tag="pv", name=f"pv{qt}_{i}")
                    for i in range(4)
                ]

            alloc_pv(0)
            for nt in range(NT):
                emit_x_load(nt)
                if nt == 0:
                    # after x(nt0) so the first K matmuls start ASAP
                    nc.sync.dma_start(out=wqkv_sb[:], in_=wqkv_d[:])
                emit_qk(nt, 0, 1)  # k pair 0
                emit_qk(nt, 1, 1)  # k pair 1
                for m in range(4):
                    emit_v(nt, m)
                if nt == 0:
                    emit_qk(0, 0, 0)  # q pair 0 (qt 0)
                    emit_qk(0, 1, 0)  # q pair 1
                if nt == 1:
                    emit_qk(1, 0, 0)  # q for qt 1 (the qt loop emits qt+1)
                    emit_qk(1, 1, 0)
                for kt in range(4 * nt, 4 * nt + 4):
                    attn_slot(0, kt, pv_tiles[0])

            # weights for the projection tail load behind the lead-in
            nc.sync.dma_start(out=wproj_sb[:], in_=wproj_d[:])
            nc.sync.dma_start(
                out=bias_sb, in_=bias_d[:].rearrange("(g p) -> p g", p=128)
            )

            emit_norm(0, pv_tiles[0])
            emit_ag(0)

            # proj chunks run with a TWO-qt lag so their matmuls never reach
            # the PE queue head before their AllGather has completed (a proj
            # matmul waiting on a collective would head-of-line-block every
            # later PE instruction).
            for qt in range(1, NT):
                alloc_pv(qt)
                for kt in range(KT):
                    attn_slot(qt, kt, pv_tiles[qt])
                    if kt == 4 and qt < NT - 1:
                        emit_qk(qt + 1, 0, 0)  # q for next qt
                    if kt == 6 and qt < NT - 1:
                        emit_qk(qt + 1, 1, 0)
                    if qt >= 2:
                        # qt==2 runs proj(0); qt==3 runs proj(1) mid-loop.
                        # placement at kt>=8 guarantees the AllGather (done
                        # ~20us into the NEXT qt at worst) has completed, so
                        # the proj matmuls never head-of-line-block the PE.
                        if kt == 8:
                            emit_proj_rhs(qt - 2)
                        if kt == 10:
                            emit_proj_mg(qt - 2, 0)
                        if kt == 12:
                            emit_proj_mg(qt - 2, 1)
                    if qt == NT - 1 and kt == 14:
                        emit_proj_rhs(qt - 1)
                if qt < NT - 1:
                    emit_norm(qt, pv_tiles[qt])
                    emit_ag(qt)
                else:
                    # tail: norm(3)+AG(3) run on ACT/DVE/DMA/cc while the PE
                    # finishes proj(2); proj(3) follows once AG(3) lands.
                    emit_norm(qt, pv_tiles[qt])
                    emit_ag(qt)
                    emit_proj_mg(qt - 1, 0)
                    emit_proj_mg(qt - 1, 1)
            # final chunk: split the gathered-rhs DMA so the projection
            # matmuls start as soon as the first half lands
            ch = NT - 1
            rhs_t = prhs_pool.tile([128, ET, 512], BF16, tag="prhs")
            for half in range(2):
                e_sl = slice(half * 4, (half + 1) * 4)
                nc.gpsimd.dma_start(
                    out=rhs_t[:, e_sl, :],
                    in_=og_ch[ch][half * 512 : (half + 1) * 512, :].rearrange(
                        "(k p) n -> p k n", p=128
                    ),
                )
            pps = [
                eps_pool.tile([128, 512], F32, tag="e", name=f"ppst{i}")
                for i in range(2)
            ]
            for half in range(2):
                for kt in range(half * 4, half * 4 + 4):
                    for mg in range(2):
                        nc.tensor.matmul(
                            pps[mg][:],
                            lhsT=wproj_sb[:, kt, mg * 128 : (mg + 1) * 128],
                            rhs=rhs_t[:, kt, :],
                            start=(kt == 0),
                            stop=(kt == ET - 1),
                        )
            for mg in range(2):
                ob = outp.tile([128, 512], BF16, tag="ob")
                nc.vector.tensor_scalar(
                    ob[:],
                    pps[mg][:],
                    bias_sb[:, mg : mg + 1],
                    0.0,
                    mybir.AluOpType.add,
                    mybir.AluOpType.max,
                )
                nc.sync.dma_start(
                    out=out_d[
                        mg * 128 : (mg + 1) * 128, ch * 512 : (ch + 1) * 512
                    ],
                    in_=ob[:],
                )

    nc.compile()
    return nc


def _get_nc():
    global _CACHED_NC
    if _CACHED_NC is None:
        _CACHED_NC = _build()
    return _CACHED_NC


def _prep_inputs(x, w_qkv, w_proj, b_proj):
    """Shard + relayout the full inputs for the 8 cores."""
    x = np.asarray(x, dtype=np.float32)
    w_qkv = np.asarray(w_qkv, dtype=np.float32)
    w_proj = np.asarray(w_proj, dtype=np.float32)
    b_proj = np.asarray(b_proj, dtype=np.float32)

    # x^T per batch re-laid out as [NT, 128, ET, 512] so each n-tile loads
    # as one contiguous DMA burst
    xts = [
        np.ascontiguousarray(
            x[b].T.reshape(ET, 128, NT, 512).transpose(2, 1, 0, 3)
        ).astype(np.float16)
        for b in range(B)
    ]
    # w_qkv rows are (h, d, qkv)-interleaved with qkv innermost
    wr = w_qkv.reshape(H, D, 3, E)
    # fold the post-softmax 1/sqrt(E) scaling into w_proj
    wp = w_proj / np.sqrt(E).astype(np.float32)

    wqkv_shards, wproj_shards, bias_shards = [], [], []
    for r in range(4):
        heads = range(4 * r, 4 * r + 4)
        qrows = np.concatenate([wr[h, :, 0, :] for h in heads], 0)  # [256, E]
        krows = np.concatenate([wr[h, :, 1, :] for h in heads], 0)
        vrows = np.concatenate([wr[h, :, 2, :] for h in heads], 0)
        w_core = np.concatenate([qrows, krows, vrows], 0)  # [768, E]
        wqkv_shards.append(
            np.ascontiguousarray(
                w_core.T.reshape(ET, 128, QKV_F).transpose(1, 0, 2)
            ).astype(np.float16)
        )
        wproj_shards.append(
            np.ascontiguousarray(
                wp[r * FC : (r + 1) * FC, :].T.reshape(ET, 128, FC).transpose(
                    1, 0, 2
                )
            ).astype(ml_dtypes.bfloat16)
        )
        bias_shards.append(np.ascontiguousarray(b_proj[r * FC : (r + 1) * FC]))

    in_maps = []
    for c in range(NCORES):
        b, r = c // 4, c % 4
        in_maps.append(
            {
                "xt": xts[b],
                "wqkvt": wqkv_shards[r],
                "wprojt": wproj_shards[r],
                "bias": bias_shards[r],
            }
        )
    return in_maps


def kernel(x, w_qkv, w_proj, b_proj):
    global LAST_EXEC_NS, LAST_RESULTS
    nc = _get_nc()
    in_maps = _prep_inputs(x, w_qkv, w_proj, b_proj)
    trace = bool(int(os.environ.get("BASS_KERNEL_TRACE", "0")))
    res = run_bass_kernel_spmd(
        nc, in_maps, list(range(NCORES)), trace=trace
    )
    LAST_EXEC_NS = res.exec_time_ns
    LAST_RESULTS = res

    out = np.empty((B, N, E), dtype=np.float32)
    for g in range(B):
        pt = np.concatenate(
            [
                res.results[4 * g + r]["out"].astype(np.float32)
                for r in range(4)
            ],
            axis=0,
        )  # [1024 f, 2048 n]
        out[g] = pt.T
    return out

